# revision 20
# baseline (speedup 1.0000x reference)
"""GAT-style attention layer (gnn_message_passing) on 8 trn2 NeuronCores.

Strategy:
  - Host: pure re-layout only (pad, transpose x, build int32 [self|neigh] index
    table, reshape weights). No FLOPs on host.
  - Phase 1 (on device, replicated per core): t_aug = x @ W_aug where
    W_aug = [Wcat | w_dst | w_src] so each node row of the DRAM table holds
    [t (128 f32), s_dst (4), s_src (4)] = 136 f32 = 544 B.
  - Phase 2 (sharded, 1/8 nodes per core): indirect-DMA gather of 17 rows
    (self + 16 neighbors) per node (one 128-row gather instruction per
    neighbor slot - walrus supports one index per partition per instruction);
    softmax(exp(leaky_relu(score))) over the 17 entries; weighted feature sum
    as per-edge DVE scaling + PE identity-matmul PSUM accumulation; ELU.
  - Node-tiles processed in pairs so the narrow elementwise/softmax ops run
    at double width (half the instruction count).
"""

import functools

import numpy as np

# Problem constants (hardcoded per harness contract).
N = 100000
DEG = 16
F_IN = 128
F_OUT = 32
H = 4
NCORES = 8
P = 128
DEG1 = DEG + 1              # 17 (self + neighbors)
ROW = F_IN + 2 * H          # 136 f32 per table row
TILES_PER_CORE = 98
SHARD = TILES_PER_CORE * P  # 12544
NPAD = SHARD * NCORES       # 100352
XT_BLK = 8                  # node-tiles per phase-1 load/store batch


def build_nc(n_pad=NPAD, n_cores=NCORES, deg1=DEG1, f_in=F_IN, f_out=F_OUT,
             h=H, xt_blk=XT_BLK, enable_asserts=False, debug_outputs=False,
             rep_p1=1, rep_p2=1, p1_mode="full", p2_mode="full"):
    """Build + compile the single-program SPMD kernel (same program all cores).

    rep_p1/rep_p2: wrap phase 1 / phase 2 in a hardware For_i loop running the
    phase that many times - used only for wall-clock timing amplification.
    """
    import concourse.bass as bass
    import concourse.tile as tile
    from concourse import bacc, mybir
    from concourse.masks import make_identity
    from contextlib import nullcontext

    f32 = mybir.dt.float32
    i32 = mybir.dt.int32
    ALU = mybir.AluOpType
    ACTF = mybir.ActivationFunctionType

    row = f_in + 2 * h
    shard = n_pad // n_cores
    assert shard % (2 * P) == 0 and n_pad % (P * xt_blk) == 0
    n_t_tiles = n_pad // P
    n_g_tiles = shard // P

    nc = bacc.Bacc(
        "TRN2",
        target_bir_lowering=False,
        debug=False,
        enable_asserts=enable_asserts,
        num_devices=n_cores,
    )

    xT_d = nc.dram_tensor("xT", [f_in, n_pad], f32, kind="ExternalInput").ap()
    idx_d = nc.dram_tensor("idx", [shard, deg1], i32, kind="ExternalInput").ap()
    ws_d = nc.dram_tensor("ws", [h * f_out, f_in], f32, kind="ExternalInput").ap()
    asrc_d = nc.dram_tensor("asrc", [h, f_out], f32, kind="ExternalInput").ap()
    adst_d = nc.dram_tensor("adst", [h, f_out], f32, kind="ExternalInput").ap()
    out_d = nc.dram_tensor("out", [shard, f_in], f32, kind="ExternalOutput").ap()
    tbl_d = nc.dram_tensor("tbl", [n_pad, row], f32, kind="Internal").ap()
    if debug_outputs:
        dbg_tbl = nc.dram_tensor("dbg_tbl", [n_pad, row], f32,
                                 kind="ExternalOutput").ap()

    with tile.TileContext(nc) as tc:
        with tc.tile_pool(name="const", bufs=1) as constp:
            ident = constp.tile([P, P], f32)
            make_identity(nc, ident[:])

            # Wsb[h*f_out + o, i] = Ws[h, o, i]
            wsb = constp.tile([P, f_in], f32)
            nc.sync.dma_start(wsb[:], ws_d[:, :])

            # rhs_build = [I_128 | a_dst cols | a_src cols]
            rhsb = constp.tile([P, row], f32)
            nc.gpsimd.memset(rhsb[:], 0.0)
            make_identity(nc, rhsb[:, 0:f_in], nomemset=True)
            for hh in range(h):
                nc.sync.dma_start(
                    rhsb[hh * f_out:(hh + 1) * f_out, f_in + hh:f_in + hh + 1],
                    adst_d[hh, :, None],
                )
                nc.sync.dma_start(
                    rhsb[hh * f_out:(hh + 1) * f_out, f_in + h + hh:f_in + h + hh + 1],
                    asrc_d[hh, :, None],
                )

            # W_aug = Wsb.T @ rhs_build : [f_in, row]
            waug = constp.tile([P, row], f32)
            with tc.tile_pool(name="wps", bufs=1, space="PSUM") as wpp:
                wps = wpp.tile([P, row], f32)
                nc.tensor.matmul(wps[:], lhsT=wsb[:], rhs=rhsb[:],
                                 start=True, stop=True)
                nc.vector.tensor_copy(waug[:], wps[:])

            # ---- Phase 1: build the full t table (replicated) ----
            loop1 = tc.For_i(0, rep_p1, 1) if rep_p1 != 1 else nullcontext()
            with loop1:
                with tc.tile_pool(name="p1x", bufs=3) as p1x, \
                     tc.tile_pool(name="p1t", bufs=3) as p1t, \
                     tc.tile_pool(name="p1ps", bufs=4, space="PSUM") as p1ps:
                    for jb in range(n_t_tiles // xt_blk):
                        xt = p1x.tile([P, P * xt_blk], f32)
                        nc.sync.dma_start(
                            xt[:], xT_d[:, jb * P * xt_blk:(jb + 1) * P * xt_blk])
                        if p1_mode == "load":
                            tiny = p1t.tile([P, 1], f32)
                            nc.vector.tensor_reduce(
                                tiny[:], xt[:, 0:P], axis=mybir.AxisListType.X,
                                op=ALU.max)
                            nc.sync.dma_start(tbl_d[jb * P:jb * P + P, 0:1],
                                              tiny[:])
                            continue
                        wide = p1t.tile([P, xt_blk * row], f32)
                        for k in range(xt_blk):
                            ps = p1ps.tile([P, row], f32)
                            nc.tensor.matmul(ps[:], lhsT=xt[:, k * P:(k + 1) * P],
                                             rhs=waug[:], start=True, stop=True)
                            nc.vector.tensor_copy(
                                wide[:, k * row:(k + 1) * row], ps[:])
                        dst = tbl_d[jb * P * xt_blk:(jb + 1) * P * xt_blk, :] \
                            .rearrange("(b p) f -> p b f", p=P)
                        nc.sync.dma_start(dst, wide[:].rearrange(
                            "p (b f) -> p b f", b=xt_blk))
                        if debug_outputs:
                            ddst = dbg_tbl[jb * P * xt_blk:(jb + 1) * P * xt_blk, :] \
                                .rearrange("(b p) f -> p b f", p=P)
                            nc.sync.dma_start(ddst, wide[:].rearrange(
                                "p (b f) -> p b f", b=xt_blk))

            # ---- Phase 2: gather + attention, node-tiles in pairs ----
            loop2 = tc.For_i(0, rep_p2, 1) if rep_p2 != 1 else nullcontext()
            with loop2:
                with tc.tile_pool(name="pidx", bufs=3) as pidx, \
                     tc.tile_pool(name="pg", bufs=2) as pg, \
                     tc.tile_pool(name="psc", bufs=3) as psc, \
                     tc.tile_pool(name="pprod", bufs=2) as pprod, \
                     tc.tile_pool(name="po", bufs=3) as po, \
                     tc.tile_pool(name="ps2", bufs=3, space="PSUM") as ps2:
                    for jp in range(n_g_tiles // 2):
                        r0 = jp * 2 * P
                        idxt = pidx.tile([P, 2 * deg1], i32)
                        nc.sync.dma_start(
                            idxt[:].rearrange("p (b d) -> p b d", b=2),
                            idx_d[r0:r0 + 2 * P, :].rearrange(
                                "(b p) d -> p b d", p=P))

                        # G[p, (b*deg1+d)*row : ...] = tbl[idx[p, b, d], :]
                        # (indices are global node ids, so this works
                        # unchanged on every core of the SPMD program)
                        G = pg.tile([P, 2 * deg1 * row], f32)
                        gather_w = row // 2 if p2_mode == "gatherhalf" else row
                        for b in range(2):
                            for d in range(deg1):
                                s = b * deg1 + d
                                nc.gpsimd.indirect_dma_start(
                                    out=G[:, s * row:s * row + gather_w],
                                    out_offset=None,
                                    in_=tbl_d[:, :],
                                    in_offset=bass.IndirectOffsetOnAxis(
                                        ap=idxt[:, s:s + 1], axis=0),
                                )

                        if p2_mode in ("gather", "gatherhalf"):
                            tiny = psc.tile([P, 2 * deg1], f32)
                            nc.vector.tensor_reduce(
                                tiny[:],
                                G[:].rearrange("p (s f) -> p s f",
                                               s=2 * deg1)[:, :, 0:1],
                                axis=mybir.AxisListType.X, op=ALU.max)
                            nc.sync.dma_start(out_d[r0:r0 + P, 0:2 * deg1],
                                              tiny[:])
                            continue
                        g0 = G[:]
                        # scores sc[p, b, hh, d] (layout b*68 + hh*17 + d)
                        sc = psc.tile([P, 2 * h * deg1], f32)
                        sc_v = sc[:].rearrange(
                            "p (b hh d) -> p b d hh", b=2, hh=h)
                        sdst = g0.rearrange(
                            "p (b d f) -> p b d f", b=2, d=deg1)[:, :, :, f_in:f_in + h]
                        bp = g0.rearrange(
                            "p (b s f) -> p b s f", b=2, s=deg1)[:, :, 0:1,
                                                                f_in + h:f_in + 2 * h]
                        ssrc = bp.to_broadcast([P, 2, deg1, h])
                        nc.vector.tensor_tensor(out=sc_v, in0=sdst, in1=ssrc,
                                                op=ALU.add)

                        # e2 = exp(exp(leaky_relu(sc, 0.2)))
                        lr = psc.tile([P, 2 * h * deg1], f32)
                        nc.vector.scalar_tensor_tensor(
                            out=lr[:], in0=sc[:], scalar=0.2, in1=sc[:],
                            op0=ALU.mult, op1=ALU.max)
                        e1 = psc.tile([P, 2 * h * deg1], f32)
                        nc.scalar.activation(e1[:], lr[:], ACTF.Exp)
                        e2 = psc.tile([P, 2 * h * deg1], f32)
                        nc.scalar.activation(e2[:], e1[:], ACTF.Exp)

                        # softmax denominators + reciprocal
                        S = psc.tile([P, 2 * h], f32)
                        nc.vector.tensor_reduce(
                            S[:], e2[:].rearrange("p (bh d) -> p bh d", d=deg1),
                            axis=mybir.AxisListType.X, op=ALU.add)
                        R = psc.tile([P, 2 * h], f32)
                        nc.vector.reciprocal(R[:], S[:])

                        # alphas (same layout as e2)
                        A = psc.tile([P, 2 * h * deg1], f32)
                        nc.vector.tensor_tensor(
                            out=A[:].rearrange("p (bh d) -> p bh d", d=deg1),
                            in0=e2[:].rearrange("p (bh d) -> p bh d", d=deg1),
                            in1=R[:].unsqueeze(2).to_broadcast([P, 2 * h, deg1]),
                            op=ALU.mult)

                        # per-edge scaled features Pt[p, d, b, hh, o]
                        # (d-major so one [128, 256] f32r matmul per d covers
                        # both tiles of the pair at 1 cycle/row)
                        Pt = pprod.tile([P, 2 * deg1 * f_in], f32)
                        g4 = g0.rearrange(
                            "p (b d f) -> p b d f", b=2, d=deg1)[:, :, :, 0:f_in] \
                            .rearrange("p b d (hh o) -> p b d hh o", hh=h) \
                            .transpose([0, 2, 1, 3, 4])
                        a4 = A[:].rearrange(
                            "p (b hh d) -> p d b hh", b=2, hh=h) \
                            .unsqueeze(4).to_broadcast([P, deg1, 2, h, f_out])
                        nc.vector.tensor_tensor(
                            out=Pt[:].rearrange(
                                "p (d b hh o) -> p d b hh o", b=2, d=deg1, hh=h),
                            in0=g4, in1=a4, op=ALU.mult)

                        # sum over d: identity-stationary accumulating matmuls
                        pso = ps2.tile([P, 2 * P], f32)
                        for d in range(deg1):
                            nc.tensor.matmul(
                                pso[:],
                                lhsT=ident[:],
                                rhs=Pt[:, d * 2 * f_in:(d + 1) * 2 * f_in],
                                start=(d == 0), stop=(d == deg1 - 1))

                        # elu(x) = max(x,0) + exp(min(x,0)) - 1
                        relu_t = po.tile([P, 2 * P], f32)
                        nc.scalar.activation(relu_t[:], pso[:], ACTF.Relu)
                        negm = po.tile([P, 2 * P], f32)
                        nc.scalar.activation(negm[:], pso[:], ACTF.Relu, scale=-1.0)
                        expm = po.tile([P, 2 * P], f32)
                        nc.scalar.activation(expm[:], negm[:], ACTF.Exp, scale=-1.0)
                        fin = po.tile([P, 2 * P], f32)
                        nc.vector.scalar_tensor_tensor(
                            out=fin[:], in0=expm[:], scalar=-1.0, in1=relu_t[:],
                            op0=ALU.add, op1=ALU.add)

                        nc.sync.dma_start(
                            out_d[r0:r0 + 2 * P, :].rearrange(
                                "(b p) f -> p b f", p=P),
                            fin[:].rearrange("p (b f) -> p b f", b=2))

    nc.compile()
    return nc


@functools.lru_cache(maxsize=1)
def _get_nc():
    return build_nc()


def prep_inputs(x, neigh_idx, Ws, As, n_pad=NPAD, n_cores=NCORES):
    """Host-side re-layout (no math): pad/transpose/index-table/reshape."""
    n = x.shape[0]
    deg = neigh_idx.shape[1]
    h = As.shape[0]
    f_out = As.shape[1] // 2
    f_in = x.shape[1]

    x = np.asarray(x, dtype=np.float32)
    x_pad = np.zeros((n_pad, f_in), dtype=np.float32)
    x_pad[:n] = x
    xT = np.ascontiguousarray(x_pad.T)                       # [f_in, n_pad]

    fullidx = np.zeros((n_pad, deg + 1), dtype=np.int32)
    fullidx[:n, 0] = np.arange(n, dtype=np.int32)
    fullidx[:n, 1:] = np.asarray(neigh_idx, dtype=np.int32)

    ws = np.ascontiguousarray(
        np.asarray(Ws, dtype=np.float32).reshape(h * f_out, f_in))
    a = np.asarray(As, dtype=np.float32)[..., 0]             # [h, 2*f_out]
    asrc = np.ascontiguousarray(a[:, :f_out])
    adst = np.ascontiguousarray(a[:, f_out:])

    shard = n_pad // n_cores
    in_maps = []
    for c in range(n_cores):
        in_maps.append({
            "xT": xT,
            "idx": np.ascontiguousarray(fullidx[c * shard:(c + 1) * shard]),
            "ws": ws,
            "asrc": asrc,
            "adst": adst,
        })
    return in_maps


def kernel(x, neigh_idx, Ws, As):
    from concourse.bass_utils import run_bass_kernel_spmd

    in_maps = prep_inputs(x, neigh_idx, Ws, As)
    nc = _get_nc()
    res = run_bass_kernel_spmd(nc, in_maps, core_ids=list(range(NCORES)))
    out = np.concatenate([res.results[c]["out"] for c in range(NCORES)], axis=0)
    return np.ascontiguousarray(out[:N]).astype(np.float32)
